# revision 1
# baseline (speedup 1.0000x reference)
"""Bass/Trainium2 kernel for nn_CasualSelfAttention (B=4, T=2048, D=1024, H=16, dk=64).

Single-core design: the axon tunnel serializes per-core NEFF dispatch and
executions across cores (measured: n replicas of the same NEFF take n x the
wall time), so one core running the whole problem minimizes wall clock. All
tensor I/O crosses the tunnel as ONE packed fp16 buffer — a single large
buffer streams at full tunnel bandwidth (~1.7ms/call) where 13 separate
buffers pay ~3.4ms of per-call latency.

The problem runs as 8 (batch x head-group) slices inside one NEFF. fp16
everywhere on the wire and in SBUF (same 10-bit mantissa as f32r/tf32), f32
PSUM. Per slice: QKV projections; scores = K^T.Q via 64-row-quadrant-packed
matmuls; exp on ACT (scale=1/8) into fp16; PV with a ones-augmented V (M=65)
for softmax sums; DVE reciprocal + K=1 broadcast matmul normalization. Each
PV matmul is deferred four s-chunks behind its exp so PE never waits on the
ACT semaphore. WO accumulates all 8 head-pairs of a batch into one PSUM
chain (full output on device, no host reduction); WO chunks for batch b are
interleaved into batch b+1's projection phase. Biases: bq/bk on device;
WV_b/WO_b folded into a host-side bias vector (attention rows sum to 1).

Packed input layout ([16512, 2048] fp16 rows):
  [    0, 4096)  xq^T  (batch-major, [b*1024+d, t])
  [ 4096, 8192)  xk^T
  [ 8192,12288)  xv^T
  [12288,13312)  wq^T  cols 0:1024   ([d_in, h*dk])
  [13312,14336)  wk^T  cols 0:1024
  [14336,15360)  wv^T  cols 0:1024
  [15360,16384)  wo^T  cols 0:1024   ([h*dk, d_out])
  [16384,16512)  cols 0:8 bq as [128, 8 pairs]; cols 8:16 bk
"""
import sys
import os

sys.path.insert(0, '/opt/trn_rl_repo')

import numpy as np
import ml_dtypes
import orjson

import concourse.bass as bass
import concourse.tile as tile
import concourse.mybir as mybir
from concourse.bass_utils import run_bass_kernel_spmd

# ---------------------------------------------------------------- waitsplit
# The walrus build in this container accepts at most ONE semaphore wait per
# engine instruction.  Tile emits multi-wait sync_info; split the extras into
# single-wait NoOps on the same engine stream (in-order => semantically equal).
_ws_counter = [0]


# PE also executes strictly in order on a single array: a PE instruction's
# wait on the PE completion semaphore (psum WAR vs an older matmul) is always
# already satisfied — dropping it removes ~2k NoOps from the critical PE
# stream plus their semaphore-propagation stalls.
_SELF_WAIT_ENGINES = ("Activation", "DVE", "PE")


def _split_instruction_waits(inst, out_list):
    si = inst.get("sync_info")
    if not si or not si.get("on_wait"):
        out_list.append(inst)
        return
    waits = si["on_wait"]
    # ACT/DVE execute strictly in order, so a compute instruction's wait on
    # its OWN engine's semaphore (slot-reuse WAW vs an older instruction on
    # the same engine) is always already satisfied — drop it instead of
    # spending a NoOp dispatch on the bottleneck ACT stream.
    eng = inst.get("engine")
    if (eng in _SELF_WAIT_ENGINES
            and inst.get("opcode") not in ("Drain", "EventSemaphore", "NoOp")):
        kept = [w for w in waits
                if w.get("ant_name", "").rsplit("_", 1)[0] != eng]
        if kept != waits:
            si = dict(si)
            si["on_wait"] = kept
            inst = dict(inst)
            inst["sync_info"] = si
            waits = kept
    if len(waits) <= 1:
        out_list.append(inst)
        return
    for w in waits[:-1]:
        _ws_counter[0] += 1
        out_list.append({
            "debug": inst.get("debug", 0),
            "engine": inst.get("engine"),
            "ins": [],
            "name": f"I-wsplit-{_ws_counter[0]}",
            "opcode": "NoOp",
            "outs": [],
            "sync_info": {"on_update": [], "on_wait": [w]},
        })
    si = dict(si)
    si["on_wait"] = [waits[-1]]
    inst = dict(inst)
    inst["sync_info"] = si
    out_list.append(inst)


def fix_multiwait_json(bir_bytes):
    d = orjson.loads(bir_bytes)
    for fn in d["functions"]:
        for bb in fn["blocks"]:
            new = []
            for inst in bb["instructions"]:
                _split_instruction_waits(inst, new)
            bb["instructions"] = new
    return orjson.dumps(d)


class WaitSplitBass(bass.Bass):
    def to_json_bytes(self):
        return fix_multiwait_json(super().to_json_bytes())


# ---------------------------------------------------------------- kernel build
P = 128
B, T, D = 4, 2048, 1024
NH_LOC = 8            # heads per slice
NP = NH_LOC // 2      # head pairs per slice
NG = 2                # head groups (slices per batch)
DK = 64
DC = D // P           # 8 d_model chunks
SC = T // P           # 16 s-chunks
NTB = T // 512        # 4 t-blocks
RQ, RK, RV = 0, B * D, 2 * B * D          # packed row bases
RWQ, RWK, RWV, RWO = 12288, 13312, 14336, 15360
RBIAS = 16384
NROWS = 16512
f32 = mybir.dt.float32
f32r = mybir.dt.float32r
f16 = mybir.dt.float16
AF = mybir.ActivationFunctionType
MULT = mybir.AluOpType.mult

_nc_cache = [None]


def build_nc():
    if _nc_cache[0] is not None:
        return _nc_cache[0]
    nc = WaitSplitBass()
    packed = nc.dram_tensor("packed", [NROWS, T], f16, kind="ExternalInput")
    out = nc.dram_tensor("out", [B * T, D], f16, kind="ExternalOutput")

    with tile.TileContext(nc) as tc:
        with tc.tile_pool(name="persist", bufs=1) as persist, \
             tc.tile_pool(name="xpool", bufs=9) as xpool, \
             tc.tile_pool(name="ppool", bufs=8) as ppool, \
             tc.tile_pool(name="rbpool", bufs=2) as rbpool, \
             tc.tile_pool(name="opool", bufs=5) as opool, \
             tc.tile_pool(name="psS", bufs=2, space="PSUM") as psS, \
             tc.tile_pool(name="psProj", bufs=2, space="PSUM") as psProj, \
             tc.tile_pool(name="psPV", bufs=1, space="PSUM") as psPV:

            # ---- persistent tiles (reused by every slice) ----
            qT2 = [persist.tile([P, T], f16, name=f"qT2_{p}") for p in range(NP)]
            kT2 = [persist.tile([P, T], f16, name=f"kT2_{p}") for p in range(NP)]
            V_aug = persist.tile([P, SC, NH_LOC, 65], f16, name="V_aug")
            nc.vector.memset(V_aug[:, :, :, 64], 1.0)
            # ctx for the whole batch: [pair-group g][t-block tb][pair p]
            ctx = [[[persist.tile([P, 512], f16, name=f"ctx_{g}_{tb}_{p}")
                     for p in range(NP)] for tb in range(NTB)] for g in range(NG)]
            wq = persist.tile([P, DC, 512], f16, name="wq")
            wk = persist.tile([P, DC, 512], f16, name="wk")
            wv = persist.tile([P, DC, 512], f16, name="wv")
            wo = persist.tile([P, NG * NP, D], f16, name="wo")
            for pg in range(NG * NP):
                nc.sync.dma_start(wo[:, pg], packed[RWO + pg * P:RWO + (pg + 1) * P, 0:D])
            b16 = persist.tile([P, 16], f16, name="b16")
            nc.sync.dma_start(b16[:], packed[RBIAS:RBIAS + P, 0:16])
            bqk = persist.tile([P, 16], f32, name="bqk")
            nc.vector.tensor_copy(bqk[:], b16[:])
            bq_s = bqk[:, 0:8]
            bk_s = bqk[:, 8:16]
            ones64 = persist.tile([1, 64], f32r, name="ones64")
            nc.vector.memset(ones64[:].bitcast(f32), 1.0)

            def alloc4(stem):
                ps = [psS.tile([P, 1024], f32, tag="scores",
                               name=f"{stem}_s{j}")[:, 0:512] for j in range(2)]
                ps += [psProj.tile([P, 512], f32, tag="proj",
                                   name=f"{stem}_p{j}") for j in range(2)]
                return ps

            def flush_evac(pend):
                # normalize pair into its ctx tile:
                # ctx[h] = pv[h][0:64] * bcast(1 / pv[h][64])
                key, pv0, pv1, ctx_p = pend
                for h, pv in ((0, pv0), (1, pv1)):
                    r_t = rbpool.tile([1, 512], f32r, tag="r",
                                      name=f"r_{key}_{h}")
                    with nc.allow_low_precision(reason="softmax recip"):
                        nc.vector.reciprocal(r_t[:], pv[64:65, :])
                    ps_rb = psProj.tile([64, 512], f32, tag="proj",
                                        name=f"ps_rb_{key}_{h}")
                    nc.tensor.matmul(ps_rb[:], ones64[:], r_t[:],
                                     start=True, stop=True)
                    rb_s = rbpool.tile([64, 512], f32, tag="rb",
                                       name=f"rb_{key}_{h}")
                    nc.vector.tensor_copy(rb_s[:], ps_rb[:])
                    nc.vector.tensor_tensor(
                        ctx_p[h * 64:(h + 1) * 64, :],
                        pv[0:64, :], rb_s[:], MULT)

            def emit_wo_chunk(wb, tb, ti, ob):
                # one [128t, 512o] output tile of batch wb, t-block tb:
                # accumulate all 8 head-pairs (both groups) in one psum chain
                ps_o = psProj.tile([P, 512], f32, tag="proj",
                                   name=f"ps_o_{wb}_{tb}_{ti}_{ob}")
                for pg in range(NG * NP):
                    nc.tensor.matmul(
                        ps_o[:], ctx[pg // NP][tb][pg % NP][:, ti * P:(ti + 1) * P],
                        wo[:, pg, ob * 512:(ob + 1) * 512],
                        start=(pg == 0), stop=(pg == NG * NP - 1))
                o_t = opool.tile([P, 512], f16, tag="o",
                                 name=f"o_{wb}_{tb}_{ti}_{ob}")
                nc.vector.tensor_copy(o_t[:], ps_o[:])
                nc.sync.dma_start(
                    out[wb * T + tb * 512 + ti * P: wb * T + tb * 512 + (ti + 1) * P,
                        ob * 512:(ob + 1) * 512], o_t[:])

            # PV work is stashed and emitted 2 s-chunks behind its exp so the
            # PE never waits on the ACT semaphore. Entries carry their own pv
            # psum pair; prev-pair entries may flush any time, current-pair
            # entries only after the prev pair's evac has been emitted.
            pv_stash = []   # (sc, p_t, pv0, pv1, p, cur_key)

            def emit_pv(entry):
                sc, p_t, pv0, pv1, p = entry
                nc.tensor.matmul(
                    pv0[:], V_aug[:, sc, 2 * p, :], p_t[:, 0:512],
                    start=(sc == 0), stop=(sc == SC - 1))
                nc.tensor.matmul(
                    pv1[:], V_aug[:, sc, 2 * p + 1, :], p_t[:, 512:1024],
                    start=(sc == 0), stop=(sc == SC - 1))

            pending = None      # un-normalized PV psum pair awaiting evac
            pending_wo = None   # batch whose WO chunks are deferred

            for sl in range(B * NG):
                b, g = sl // NG, sl % NG
                gs = slice(g * 512, (g + 1) * 512)
                # ---- phase A: QKV projections for (batch b, group g) ----
                for c in range(DC):
                    nc.sync.dma_start(wq[:, c], packed[RWQ + c * P:RWQ + (c + 1) * P, gs])
                    nc.sync.dma_start(wk[:, c], packed[RWK + c * P:RWK + (c + 1) * P, gs])
                    nc.sync.dma_start(wv[:, c], packed[RWV + c * P:RWV + (c + 1) * P, gs])
                xhold = {}   # (tag, c) -> [P, 1024] x tile spanning a tb-pair
                for tb in range(NTB):
                    ts_ = slice(tb * 512, (tb + 1) * 512)
                    off = (tb % 2) * 512
                    for qk, (w_t, rbase, b_s, dst) in enumerate(
                            ((wq, RQ, bq_s, qT2), (wk, RK, bk_s, kT2))):
                        ps4 = alloc4(f"psqk{sl}_{tb}_{qk}")
                        for c in range(DC):
                            if tb % 2 == 0:
                                x_c = xpool.tile([P, 1024], f16, tag=f"x{qk}",
                                                 name=f"x{qk}_{sl}_{tb}_{c}")
                                nc.sync.dma_start(
                                    x_c[:],
                                    packed[rbase + b * D + c * P:
                                           rbase + b * D + (c + 1) * P,
                                           tb * 512:(tb + 2) * 512])
                                xhold[(qk, c)] = x_c
                            x_c = xhold[(qk, c)]
                            for p in range(NP):
                                nc.tensor.matmul(
                                    ps4[p][:], w_t[:, c, p * P:(p + 1) * P],
                                    x_c[:, off:off + 512],
                                    start=(c == 0), stop=(c == DC - 1))
                        for p in range(NP):
                            nc.vector.tensor_scalar_add(
                                dst[p][:, ts_], ps4[p][:],
                                b_s[:, g * NP + p: g * NP + p + 1])
                    # v: 4 t-tile groups, chunk-outer
                    ps4 = alloc4(f"psv{sl}_{tb}")
                    for c in range(DC):
                        if tb % 2 == 0:
                            xv_c = xpool.tile([P, 1024], f16, tag="xv",
                                              name=f"xv_{sl}_{tb}_{c}")
                            nc.sync.dma_start(
                                xv_c[:],
                                packed[RV + b * D + c * P:
                                       RV + b * D + (c + 1) * P,
                                       tb * 512:(tb + 2) * 512])
                            xhold[("v", c)] = xv_c
                        xv_c = xhold[("v", c)]
                        for ti in range(4):
                            nc.tensor.matmul(
                                ps4[ti][:], xv_c[:, off + ti * P:off + (ti + 1) * P],
                                wv[:, c], start=(c == 0), stop=(c == DC - 1))
                    for ti in range(4):
                        tt = tb * 4 + ti
                        nc.vector.tensor_copy(
                            V_aug[:, tt, :, 0:64],
                            ps4[ti][:].rearrange("p (h d) -> p h d", d=64))
                    # previous batch's WO: 8 chunks per projection t-block
                    # (PE slack while ACT idles through phase A)
                    if pending_wo is not None:
                        for ti in range(4):
                            for ob in range(2):
                                emit_wo_chunk(pending_wo, tb, ti, ob)
                        if tb == NTB - 1:
                            pending_wo = None

                    # early attention for (t-block 0, pair 0): every dep of
                    # s-chunk quarter tb is produced by A(tb), so run it here
                    # to feed ACT during the projection phase.
                    if tb == 0:
                        pv_e0 = psPV.tile([65, 512], f32, tag="pv0",
                                          name=f"pv0_{sl}_0_0")
                        pv_e1 = psPV.tile([65, 512], f32, tag="pv1",
                                          name=f"pv1_{sl}_0_0")
                    for sc in range(4 * tb, 4 * tb + 4):
                        ss = slice(sc * P, (sc + 1) * P)
                        ps_s = psS.tile([P, 1024], f32, tag="scores",
                                        name=f"ps_se_{sl}_{sc}")
                        nc.tensor.matmul(
                            ps_s[:, 0:512], qT2[0][0:64, ss],
                            kT2[0][0:64, 0:512], start=True, stop=True,
                            tile_position=(0, 0))
                        nc.tensor.matmul(
                            ps_s[:, 512:1024], qT2[0][64:128, ss],
                            kT2[0][64:128, 0:512], start=True, stop=True,
                            tile_position=(64, 0))
                        p_t = ppool.tile([P, 1024], f16, tag="p",
                                         name=f"pe_{sl}_{sc}")
                        nc.scalar.activation(p_t[:], ps_s[:], AF.Exp,
                                             scale=0.125)
                        pv_stash.append((sc, p_t, pv_e0, pv_e1, 0))
                        while len(pv_stash) > 4:
                            emit_pv(pv_stash.pop(0))

                # the early unit's evac is deferred into the next pair's
                # attention via the regular pending chain
                pending = (f"{sl}_0_0", pv_e0, pv_e1, ctx[g][0][0])

                # ---- phase B: attention for the remaining 15 pairs ----
                for tb in range(NTB):
                    ts_ = slice(tb * 512, (tb + 1) * 512)
                    for p in range(NP):
                        if tb == 0 and p == 0:
                            continue
                        pv0 = psPV.tile([65, 512], f32, tag="pv0",
                                        name=f"pv0_{sl}_{tb}_{p}")
                        pv1 = psPV.tile([65, 512], f32, tag="pv1",
                                        name=f"pv1_{sl}_{tb}_{p}")
                        for sc in range(SC):
                            ss = slice(sc * P, (sc + 1) * P)
                            ps_s = psS.tile([P, 1024], f32, tag="scores",
                                            name=f"ps_s_{sl}_{tb}_{p}_{sc}")
                            nc.tensor.matmul(
                                ps_s[:, 0:512], qT2[p][0:64, ss],
                                kT2[p][0:64, ts_], start=True, stop=True,
                                tile_position=(0, 0))
                            nc.tensor.matmul(
                                ps_s[:, 512:1024], qT2[p][64:128, ss],
                                kT2[p][64:128, ts_], start=True, stop=True,
                                tile_position=(64, 0))
                            p_t = ppool.tile([P, 1024], f16, tag="p",
                                             name=f"p_{sl}_{tb}_{p}_{sc}")
                            nc.scalar.activation(p_t[:], ps_s[:], AF.Exp,
                                                 scale=0.125)
                            pv_stash.append((sc, p_t, pv0, pv1, p))
                            if pending is not None:
                                if sc < 2:
                                    # flush only prev-pair entries so ACT gets
                                    # a 2-exp head start before evac DVE work
                                    while pv_stash and pv_stash[0][2] is not pv0:
                                        emit_pv(pv_stash.pop(0))
                                    continue
                                if sc == 2:
                                    flush_evac(pending)
                                    pending = None
                            while len(pv_stash) > 4:
                                emit_pv(pv_stash.pop(0))
                        pending_next = (f"{sl}_{tb}_{p}", pv0, pv1, ctx[g][tb][p])
                        if pending is not None:
                            # only reachable if evac never fired (shouldn't
                            # happen: SC > 2), but keep the chain sound
                            flush_evac(pending)
                        pending = pending_next

                # The next slice's phase A emits WO chunks that read every ctx
                # tile of this batch, so the last pair's evac cannot stay
                # deferred across the slice boundary (and its PV chain must be
                # complete before the evac reads it).
                while pv_stash:
                    emit_pv(pv_stash.pop(0))
                if pending is not None:
                    flush_evac(pending)
                    pending = None
                if g == NG - 1:
                    pending_wo = b

            # tail: last batch's WO
            if pending_wo is not None:
                for tb in range(NTB):
                    for ti in range(4):
                        for ob in range(2):
                            emit_wo_chunk(pending_wo, tb, ti, ob)
                pending_wo = None

    # NOTE: fusing the standalone Ldweights into self-loading Matmults
    # (ldweights=True) compiles and is numerically correct, but measured
    # 11.2ms/iter vs 7.2ms — the separate Ldweights overlaps the previous
    # matmul on the device, the self-loading form serializes. Keep the pairs.
    _nc_cache[0] = nc
    return nc


# ---------------------------------------------------------------- host side
def make_in_maps(keys, queries, values, WK_w, WK_b, WQ_w, WQ_b, WV_w, WV_b, WO_w):
    packed = np.zeros((NROWS, T), np.float16)

    def xT(dst_base, x):  # [B,T,D] f32 -> rows [dst_base:dst_base+B*D] as [D, T] per batch
        x16 = np.asarray(x, dtype=np.float32).astype(np.float16)
        packed[dst_base:dst_base + B * D] = (
            x16.transpose(0, 2, 1).reshape(B * D, T))

    xT(RQ, queries)
    xT(RK, keys)
    xT(RV, values)
    packed[RWQ:RWQ + D, 0:D] = np.asarray(WQ_w, np.float32).astype(np.float16).T
    packed[RWK:RWK + D, 0:D] = np.asarray(WK_w, np.float32).astype(np.float16).T
    packed[RWV:RWV + D, 0:D] = np.asarray(WV_w, np.float32).astype(np.float16).T
    packed[RWO:RWO + D, 0:D] = np.asarray(WO_w, np.float32).astype(np.float16).T
    packed[RBIAS:RBIAS + P, 0:8] = (
        np.asarray(WQ_b, np.float32).astype(np.float16).reshape(NG * NP, P).T)
    packed[RBIAS:RBIAS + P, 8:16] = (
        np.asarray(WK_b, np.float32).astype(np.float16).reshape(NG * NP, P).T)
    return [{"packed": packed}]


def kernel(keys, queries, values, pad_mask, WK_w, WK_b, WQ_w, WQ_b, WV_w, WV_b,
           WO_w, WO_b):
    nc = build_nc()
    in_maps = make_in_maps(keys, queries, values, WK_w, WK_b, WQ_w, WQ_b,
                           WV_w, WV_b, WO_w)
    res = run_bass_kernel_spmd(nc, in_maps, [0])
    # free-dim biases folded on host: WO_b directly; WV_b exactly via
    # WV_b @ WO_w^T (attention rows sum to 1).
    bias = (np.asarray(WO_b, np.float64)
            + np.asarray(WV_b, np.float64) @ np.asarray(WO_w, np.float64).T)
    out = (res.results[0]["out"].astype(np.float64).reshape(B, T, D)
           + bias).astype(np.float32)
    return out



# revision 2
# speedup vs baseline: 6.2307x; 6.2307x over previous
"""Bass/Trainium2 kernel for nn_CasualSelfAttention (B=4, T=2048, D=1024, H=16, dk=64).

8-core SPMD design, sharded by (batch x output-time-half). The reference's
softmax contracts the QUERY index s (scores[t,s] = k_t.q_s, softmax over s,
ctx[t] = sum_s attn[t,s] v[s]), so the output row index t comes from K.
Each core owns (batch b, t-half h): it computes all 16 heads for its 1024
output rows, needing xk only for its t-half but full-T xq and xv (the
softmax-side projections are duplicated across the pair of cores sharing a
batch — 19% extra MACs, zero cross-core communication; each core writes a
disjoint [1024, 1024] slab of the final output).

Per core: 2 sequential head-group slices (8 heads each) reusing one set of
persistent tiles. fp16 on the wire and in SBUF, f32 PSUM. Per slice: QKV
projections; scores = Q^T.K via 64-row-quadrant-packed matmuls; exp on ACT
(scale=1/8) into fp16; PV with a ones-augmented V (M=65) for softmax sums;
DVE reciprocal + K=1 broadcast matmul normalization. Each PV matmul is
deferred four s-chunks behind its exp so PE never waits on the ACT
semaphore. WO accumulates all 8 head-pairs into one PSUM chain per output
chunk, emitted after both slices. Biases: bq/bk on device; WV_b/WO_b folded
into a host-side bias vector (attention rows sum to 1).

Packed per-core input layout ([5248, 2048] fp16 rows):
  [    0, 1024)  xq^T  batch b          ([d, s], full T)
  [ 1024, 2048)  xv^T  batch b          ([d, s], full T)
  [ 2048, 3072)  xk^T  batch b, cols 0:1024 = t-half  ([d, t])
  [ 3072, 4096)  cols 0:1024 wq^T | cols 1024:2048 wk^T   ([d_in, h*dk])
  [ 4096, 5120)  cols 0:1024 wv^T | cols 1024:2048 wo^T   (wo: [h*dk, d_out])
  [ 5120, 5248)  cols 0:8 bq as [128, 8 pairs]; cols 8:16 bk
"""
import sys
import os

sys.path.insert(0, '/opt/trn_rl_repo')

import numpy as np
import ml_dtypes
import orjson

import concourse.bass as bass
import concourse.tile as tile
import concourse.mybir as mybir
from concourse.bass_utils import run_bass_kernel_spmd

# ---------------------------------------------------------------- waitsplit
# The walrus build in this container accepts at most ONE semaphore wait per
# engine instruction.  Tile emits multi-wait sync_info; split the extras into
# single-wait NoOps on the same engine stream (in-order => semantically equal).
_ws_counter = [0]


# PE also executes strictly in order on a single array: a PE instruction's
# wait on the PE completion semaphore (psum WAR vs an older matmul) is always
# already satisfied — dropping it removes NoOps from the critical PE
# stream plus their semaphore-propagation stalls.
_SELF_WAIT_ENGINES = ("Activation", "DVE", "PE")


def _split_instruction_waits(inst, out_list):
    si = inst.get("sync_info")
    if not si or not si.get("on_wait"):
        out_list.append(inst)
        return
    waits = si["on_wait"]
    # ACT/DVE execute strictly in order, so a compute instruction's wait on
    # its OWN engine's semaphore (slot-reuse WAW vs an older instruction on
    # the same engine) is always already satisfied — drop it instead of
    # spending a NoOp dispatch on the bottleneck ACT stream.
    eng = inst.get("engine")
    if (eng in _SELF_WAIT_ENGINES
            and inst.get("opcode") not in ("Drain", "EventSemaphore", "NoOp")):
        kept = [w for w in waits
                if w.get("ant_name", "").rsplit("_", 1)[0] != eng]
        if kept != waits:
            si = dict(si)
            si["on_wait"] = kept
            inst = dict(inst)
            inst["sync_info"] = si
            waits = kept
    if len(waits) <= 1:
        out_list.append(inst)
        return
    for w in waits[:-1]:
        _ws_counter[0] += 1
        out_list.append({
            "debug": inst.get("debug", 0),
            "engine": inst.get("engine"),
            "ins": [],
            "name": f"I-wsplit-{_ws_counter[0]}",
            "opcode": "NoOp",
            "outs": [],
            "sync_info": {"on_update": [], "on_wait": [w]},
        })
    si = dict(si)
    si["on_wait"] = [waits[-1]]
    inst = dict(inst)
    inst["sync_info"] = si
    out_list.append(inst)


def fix_multiwait_json(bir_bytes):
    d = orjson.loads(bir_bytes)
    for fn in d["functions"]:
        for bb in fn["blocks"]:
            new = []
            for inst in bb["instructions"]:
                _split_instruction_waits(inst, new)
            bb["instructions"] = new
    return orjson.dumps(d)


class WaitSplitBass(bass.Bass):
    def to_json_bytes(self):
        return fix_multiwait_json(super().to_json_bytes())


# ---------------------------------------------------------------- kernel build
P = 128
B, T, D = 4, 2048, 1024
TQ = T // 2           # output rows per core (t-half)
NH_LOC = 8            # heads per slice
NP = NH_LOC // 2      # head pairs per slice
NG = 2                # head groups (slices per core)
DC = D // P           # 8 d_model chunks
SC = T // P           # 16 s-chunks (softmax axis, full T)
NTB = T // 512        # 4 s-blocks for q/v projections
NTB_K = TQ // 512     # 2 t-blocks for k projection / scores / WO
RQ, RV, RK = 0, D, 2 * D                  # packed row bases
RW1, RW2, RBIAS = 3072, 4096, 5120
NROWS = 5248
NCORES = B * 2
f32 = mybir.dt.float32
f32r = mybir.dt.float32r
f16 = mybir.dt.float16
AF = mybir.ActivationFunctionType
MULT = mybir.AluOpType.mult

_nc_cache = [None]


def build_nc():
    if _nc_cache[0] is not None:
        return _nc_cache[0]
    nc = WaitSplitBass()
    packed = nc.dram_tensor("packed", [NROWS, T], f16, kind="ExternalInput")
    out = nc.dram_tensor("out", [TQ, D], f16, kind="ExternalOutput")

    with tile.TileContext(nc) as tc:
        with tc.tile_pool(name="persist", bufs=1) as persist, \
             tc.tile_pool(name="xpool", bufs=9) as xpool, \
             tc.tile_pool(name="ppool", bufs=8) as ppool, \
             tc.tile_pool(name="rbpool", bufs=2) as rbpool, \
             tc.tile_pool(name="opool", bufs=5) as opool, \
             tc.tile_pool(name="psS", bufs=2, space="PSUM") as psS, \
             tc.tile_pool(name="psProj", bufs=2, space="PSUM") as psProj, \
             tc.tile_pool(name="psPV", bufs=1, space="PSUM") as psPV:

            # ---- persistent tiles (reused by both slices) ----
            qT2 = [persist.tile([P, T], f16, name=f"qT2_{p}") for p in range(NP)]
            kT2 = [persist.tile([P, TQ], f16, name=f"kT2_{p}") for p in range(NP)]
            V_aug = persist.tile([P, SC, NH_LOC, 65], f16, name="V_aug")
            nc.vector.memset(V_aug[:, :, :, 64], 1.0)
            # ctx for the whole core: [group g][t-block tb][pair p]
            ctx = [[[persist.tile([P, 512], f16, name=f"ctx_{g}_{tb}_{p}")
                     for p in range(NP)] for tb in range(NTB_K)] for g in range(NG)]
            wq = persist.tile([P, DC, 512], f16, name="wq")
            wk = persist.tile([P, DC, 512], f16, name="wk")
            wv = persist.tile([P, DC, 512], f16, name="wv")
            wo = persist.tile([P, NG * NP, D], f16, name="wo")
            for pg in range(NG * NP):
                nc.sync.dma_start(wo[:, pg],
                                  packed[RW2 + pg * P:RW2 + (pg + 1) * P, D:2 * D])
            b16 = persist.tile([P, 16], f16, name="b16")
            nc.sync.dma_start(b16[:], packed[RBIAS:RBIAS + P, 0:16])
            bqk = persist.tile([P, 16], f32, name="bqk")
            nc.vector.tensor_copy(bqk[:], b16[:])
            bq_s = bqk[:, 0:8]
            bk_s = bqk[:, 8:16]
            ones64 = persist.tile([1, 64], f32r, name="ones64")
            nc.vector.memset(ones64[:].bitcast(f32), 1.0)

            def alloc4(stem):
                ps = [psS.tile([P, 1024], f32, tag="scores",
                               name=f"{stem}_s{j}")[:, 0:512] for j in range(2)]
                ps += [psProj.tile([P, 512], f32, tag="proj",
                                   name=f"{stem}_p{j}") for j in range(2)]
                return ps

            def flush_evac(pend):
                # normalize pair into its ctx tile:
                # ctx[h] = pv[h][0:64] * bcast(1 / pv[h][64])
                key, pv0, pv1, ctx_p = pend
                for h, pv in ((0, pv0), (1, pv1)):
                    r_t = rbpool.tile([1, 512], f32r, tag="r",
                                      name=f"r_{key}_{h}")
                    with nc.allow_low_precision(reason="softmax recip"):
                        nc.vector.reciprocal(r_t[:], pv[64:65, :])
                    ps_rb = psProj.tile([64, 512], f32, tag="proj",
                                        name=f"ps_rb_{key}_{h}")
                    nc.tensor.matmul(ps_rb[:], ones64[:], r_t[:],
                                     start=True, stop=True)
                    rb_s = rbpool.tile([64, 512], f32, tag="rb",
                                       name=f"rb_{key}_{h}")
                    nc.vector.tensor_copy(rb_s[:], ps_rb[:])
                    nc.vector.tensor_tensor(
                        ctx_p[h * 64:(h + 1) * 64, :],
                        pv[0:64, :], rb_s[:], MULT)

            def emit_wo_chunk(tb, ti, ob):
                # one [128t, 512o] output tile of t-block tb:
                # accumulate all 8 head-pairs (both groups) in one psum chain
                ps_o = psProj.tile([P, 512], f32, tag="proj",
                                   name=f"ps_o_{tb}_{ti}_{ob}")
                for pg in range(NG * NP):
                    nc.tensor.matmul(
                        ps_o[:], ctx[pg // NP][tb][pg % NP][:, ti * P:(ti + 1) * P],
                        wo[:, pg, ob * 512:(ob + 1) * 512],
                        start=(pg == 0), stop=(pg == NG * NP - 1))
                o_t = opool.tile([P, 512], f16, tag="o",
                                 name=f"o_{tb}_{ti}_{ob}")
                nc.vector.tensor_copy(o_t[:], ps_o[:])
                nc.sync.dma_start(
                    out[tb * 512 + ti * P: tb * 512 + (ti + 1) * P,
                        ob * 512:(ob + 1) * 512], o_t[:])

            # PV work is stashed and emitted a few s-chunks behind its exp so
            # the PE never waits on the ACT semaphore. Entries carry their own
            # pv psum pair; prev-pair entries may flush any time, current-pair
            # entries only after the prev pair's evac has been emitted.
            pv_stash = []   # (sc, p_t, pv0, pv1, p)

            def emit_pv(entry):
                sc, p_t, pv0, pv1, p = entry
                nc.tensor.matmul(
                    pv0[:], V_aug[:, sc, 2 * p, :], p_t[:, 0:512],
                    start=(sc == 0), stop=(sc == SC - 1))
                nc.tensor.matmul(
                    pv1[:], V_aug[:, sc, 2 * p + 1, :], p_t[:, 512:1024],
                    start=(sc == 0), stop=(sc == SC - 1))

            pending = None      # un-normalized PV psum pair awaiting evac

            for g in range(NG):
                gs = slice(g * 512, (g + 1) * 512)
                # ---- phase A: QKV projections for head group g ----
                for c in range(DC):
                    nc.sync.dma_start(
                        wq[:, c], packed[RW1 + c * P:RW1 + (c + 1) * P, gs])
                    nc.sync.dma_start(
                        wk[:, c], packed[RW1 + c * P:RW1 + (c + 1) * P,
                                         D + g * 512:D + (g + 1) * 512])
                    nc.sync.dma_start(
                        wv[:, c], packed[RW2 + c * P:RW2 + (c + 1) * P, gs])
                xhold = {}   # (tag, c) -> [P, 1024] x tile spanning a tb-pair
                for tb in range(NTB):
                    ts_ = slice(tb * 512, (tb + 1) * 512)
                    off = (tb % 2) * 512
                    # q projection (softmax side, full T: 4 t-blocks)
                    ps4 = alloc4(f"psq{g}_{tb}")
                    for c in range(DC):
                        if tb % 2 == 0:
                            x_c = xpool.tile([P, 1024], f16, tag="xq",
                                             name=f"xq_{g}_{tb}_{c}")
                            nc.sync.dma_start(
                                x_c[:],
                                packed[RQ + c * P:RQ + (c + 1) * P,
                                       tb * 512:(tb + 2) * 512])
                            xhold[("q", c)] = x_c
                        x_c = xhold[("q", c)]
                        for p in range(NP):
                            nc.tensor.matmul(
                                ps4[p][:], wq[:, c, p * P:(p + 1) * P],
                                x_c[:, off:off + 512],
                                start=(c == 0), stop=(c == DC - 1))
                    for p in range(NP):
                        nc.vector.tensor_scalar_add(
                            qT2[p][:, ts_], ps4[p][:],
                            bq_s[:, g * NP + p: g * NP + p + 1])
                    # k projection (output side, t-half only: 2 t-blocks)
                    if tb < NTB_K:
                        ps4 = alloc4(f"psk{g}_{tb}")
                        for c in range(DC):
                            if tb == 0:
                                xk_c = xpool.tile([P, 1024], f16, tag="xk",
                                                  name=f"xk_{g}_{c}")
                                nc.sync.dma_start(
                                    xk_c[:],
                                    packed[RK + c * P:RK + (c + 1) * P, 0:1024])
                                xhold[("k", c)] = xk_c
                            xk_c = xhold[("k", c)]
                            for p in range(NP):
                                nc.tensor.matmul(
                                    ps4[p][:], wk[:, c, p * P:(p + 1) * P],
                                    xk_c[:, off:off + 512],
                                    start=(c == 0), stop=(c == DC - 1))
                        for p in range(NP):
                            nc.vector.tensor_scalar_add(
                                kT2[p][:, ts_], ps4[p][:],
                                bk_s[:, g * NP + p: g * NP + p + 1])
                    # v: 4 s-tile groups, chunk-outer (full T)
                    ps4 = alloc4(f"psv{g}_{tb}")
                    for c in range(DC):
                        if tb % 2 == 0:
                            xv_c = xpool.tile([P, 1024], f16, tag="xv",
                                              name=f"xv_{g}_{tb}_{c}")
                            nc.sync.dma_start(
                                xv_c[:],
                                packed[RV + c * P:RV + (c + 1) * P,
                                       tb * 512:(tb + 2) * 512])
                            xhold[("v", c)] = xv_c
                        xv_c = xhold[("v", c)]
                        for ti in range(4):
                            nc.tensor.matmul(
                                ps4[ti][:], xv_c[:, off + ti * P:off + (ti + 1) * P],
                                wv[:, c], start=(c == 0), stop=(c == DC - 1))
                    for ti in range(4):
                        tt = tb * 4 + ti
                        nc.vector.tensor_copy(
                            V_aug[:, tt, :, 0:64],
                            ps4[ti][:].rearrange("p (h d) -> p h d", d=64))

                    # early attention for (t-block 0, pair 0): s-chunk quarter
                    # tb's deps are produced by A(tb), so run it here to feed
                    # ACT during the projection phase.
                    if tb == 0:
                        pv_e0 = psPV.tile([65, 512], f32, tag="pv0",
                                          name=f"pv0_{g}_0_0")
                        pv_e1 = psPV.tile([65, 512], f32, tag="pv1",
                                          name=f"pv1_{g}_0_0")
                    for sc in range(4 * tb, 4 * tb + 4):
                        ss = slice(sc * P, (sc + 1) * P)
                        ps_s = psS.tile([P, 1024], f32, tag="scores",
                                        name=f"ps_se_{g}_{sc}")
                        nc.tensor.matmul(
                            ps_s[:, 0:512], qT2[0][0:64, ss],
                            kT2[0][0:64, 0:512], start=True, stop=True,
                            tile_position=(0, 0))
                        nc.tensor.matmul(
                            ps_s[:, 512:1024], qT2[0][64:128, ss],
                            kT2[0][64:128, 0:512], start=True, stop=True,
                            tile_position=(64, 0))
                        p_t = ppool.tile([P, 1024], f16, tag="p",
                                         name=f"pe_{g}_{sc}")
                        nc.scalar.activation(p_t[:], ps_s[:], AF.Exp,
                                             scale=0.125)
                        pv_stash.append((sc, p_t, pv_e0, pv_e1, 0))
                        while len(pv_stash) > 4:
                            emit_pv(pv_stash.pop(0))

                # the early unit's evac is deferred into the next pair's
                # attention via the regular pending chain
                pending = (f"{g}_0_0", pv_e0, pv_e1, ctx[g][0][0])

                # ---- phase B: attention for the remaining 7 pairs ----
                for tb in range(NTB_K):
                    ts_ = slice(tb * 512, (tb + 1) * 512)
                    for p in range(NP):
                        if tb == 0 and p == 0:
                            continue
                        pv0 = psPV.tile([65, 512], f32, tag="pv0",
                                        name=f"pv0_{g}_{tb}_{p}")
                        pv1 = psPV.tile([65, 512], f32, tag="pv1",
                                        name=f"pv1_{g}_{tb}_{p}")
                        for sc in range(SC):
                            ss = slice(sc * P, (sc + 1) * P)
                            ps_s = psS.tile([P, 1024], f32, tag="scores",
                                            name=f"ps_s_{g}_{tb}_{p}_{sc}")
                            nc.tensor.matmul(
                                ps_s[:, 0:512], qT2[p][0:64, ss],
                                kT2[p][0:64, ts_], start=True, stop=True,
                                tile_position=(0, 0))
                            nc.tensor.matmul(
                                ps_s[:, 512:1024], qT2[p][64:128, ss],
                                kT2[p][64:128, ts_], start=True, stop=True,
                                tile_position=(64, 0))
                            p_t = ppool.tile([P, 1024], f16, tag="p",
                                             name=f"p_{g}_{tb}_{p}_{sc}")
                            nc.scalar.activation(p_t[:], ps_s[:], AF.Exp,
                                                 scale=0.125)
                            pv_stash.append((sc, p_t, pv0, pv1, p))
                            if pending is not None:
                                if sc < 2:
                                    # flush only prev-pair entries so ACT gets
                                    # a 2-exp head start before evac DVE work
                                    while pv_stash and pv_stash[0][2] is not pv0:
                                        emit_pv(pv_stash.pop(0))
                                    continue
                                if sc == 2:
                                    flush_evac(pending)
                                    pending = None
                            while len(pv_stash) > 4:
                                emit_pv(pv_stash.pop(0))
                        pending_next = (f"{g}_{tb}_{p}", pv0, pv1, ctx[g][tb][p])
                        if pending is not None:
                            # only reachable if evac never fired (shouldn't
                            # happen: SC > 2), but keep the chain sound
                            flush_evac(pending)
                        pending = pending_next

                # next slice's phase A reuses qT2/kT2/V_aug, so this slice's
                # attention must fully drain before those are overwritten
                # (Tile's WAR tracking handles the sync; we just order emits).
                while pv_stash:
                    emit_pv(pv_stash.pop(0))
                if pending is not None:
                    flush_evac(pending)
                    pending = None

            # tail: WO for both groups (needs all 16 heads' ctx)
            for tb in range(NTB_K):
                for ti in range(4):
                    for ob in range(2):
                        emit_wo_chunk(tb, ti, ob)

    # NOTE: fusing the standalone Ldweights into self-loading Matmults
    # (ldweights=True) compiles and is numerically correct, but measured
    # slower on the single-core variant — the separate Ldweights overlaps the
    # previous matmul on the device, the self-loading form serializes.
    _nc_cache[0] = nc
    return nc


# ---------------------------------------------------------------- host side
def make_in_maps(keys, queries, values, WK_w, WK_b, WQ_w, WQ_b, WV_w, WV_b, WO_w):
    q16 = np.asarray(queries, np.float32).astype(np.float16)   # [B,T,D]
    k16 = np.asarray(keys, np.float32).astype(np.float16)
    v16 = np.asarray(values, np.float32).astype(np.float16)
    wq16 = np.asarray(WQ_w, np.float32).astype(np.float16).T   # [d_in, h*dk]
    wk16 = np.asarray(WK_w, np.float32).astype(np.float16).T
    wv16 = np.asarray(WV_w, np.float32).astype(np.float16).T
    wo16 = np.asarray(WO_w, np.float32).astype(np.float16).T   # [h*dk, d_out]
    bq16 = np.asarray(WQ_b, np.float32).astype(np.float16).reshape(NG * NP, P).T
    bk16 = np.asarray(WK_b, np.float32).astype(np.float16).reshape(NG * NP, P).T

    in_maps = []
    for c in range(NCORES):
        b, h = c // 2, c % 2
        packed = np.zeros((NROWS, T), np.float16)
        packed[RQ:RQ + D] = q16[b].T                    # [D, T]
        packed[RV:RV + D] = v16[b].T
        packed[RK:RK + D, 0:TQ] = k16[b, h * TQ:(h + 1) * TQ].T
        packed[RW1:RW1 + D, 0:D] = wq16
        packed[RW1:RW1 + D, D:2 * D] = wk16
        packed[RW2:RW2 + D, 0:D] = wv16
        packed[RW2:RW2 + D, D:2 * D] = wo16
        packed[RBIAS:RBIAS + P, 0:8] = bq16
        packed[RBIAS:RBIAS + P, 8:16] = bk16
        in_maps.append({"packed": packed})
    return in_maps


def kernel(keys, queries, values, pad_mask, WK_w, WK_b, WQ_w, WQ_b, WV_w, WV_b,
           WO_w, WO_b):
    nc = build_nc()
    in_maps = make_in_maps(keys, queries, values, WK_w, WK_b, WQ_w, WQ_b,
                           WV_w, WV_b, WO_w)
    res = run_bass_kernel_spmd(nc, in_maps, list(range(NCORES)))
    # free-dim biases folded on host: WO_b directly; WV_b exactly via
    # WV_b @ WO_w^T (attention rows sum to 1).
    bias = (np.asarray(WO_b, np.float64)
            + np.asarray(WV_b, np.float64) @ np.asarray(WO_w, np.float64).T)
    halves = np.stack([np.asarray(res.results[c]["out"]) for c in range(NCORES)])
    out = (halves.astype(np.float64).reshape(B, T, D) + bias).astype(np.float32)
    return out
